# revision 6
# baseline (speedup 1.0000x reference)
"""Trainium2 Bass kernel for nn_CenterLossN (center-loss style reduction).

Math (per batch n, class c; H=W=384, C=11, N=32):
    res[n,c]   = x[n,c]^2 + centers[n,c]^2 - 2 * x[n,c] @ centers[n,c]
    out[n,h,w] = max_c softmax_c(res)[n,c,h,w] = 1 / sum_c exp(res_c - max_c res_c)
    loss       = sum(clip(out * labels, 1e-12, 1e12)) / (N*H*W)

Device strategy (data-parallel over N across 8 cores, 4 batches/core):
  Host ships, per (n,c) plane, three bf16 [384,384] arrays:
    xt2 = (-2*x)^T          -> matmul lhsT (PSUM gets -2*x@c directly)
    cc  = centers           -> matmul rhs
    ee  = x^2 + centers^2   -> injected into the same PSUM accumulation
                               via an identity-matmul (PSUM += I^T @ ee)
  so PSUM ends up holding s = res in fp32 with zero vector-engine work.
  Per 128-row chunk (mc): classes run in PSUM groups of 4/4/3 banks; one
  batched ACT copy drains each group to bf16 SBUF. Then DVE tree-max over
  11 classes, one broadcast-subtract, one in-place batched ACT exp,
  DVE tree-add, reciprocal, label multiply and per-partition reduce.
  clip: only label==0 hits the 1e-12 floor (1/sum >= 1/11 and <= 1);
  host adds 1e-12 * count(labels==0) exactly.
"""

import numpy as np
import ml_dtypes

N, C, H, W = 32, 11, 384, 384
N_CORES = 8
N_LOC = N // N_CORES          # 4 batches per core
PAIRS = N_LOC * C             # 44 (n,c) planes per core
MC = H // 128                 # 3 row-chunks
KC = W // 128                 # 3 contraction chunks
GROUPS = [(0, 4), (4, 4), (8, 3)]   # (first class, size) PSUM bank groups

TAIL_MODE = "recip"           # "recip" | "divide" | "gpsimd_divide"

_BF16 = ml_dtypes.bfloat16
_COMPILED = None


def _build(n_loc=N_LOC):
    from contextlib import ExitStack
    import concourse.bass as bass
    import concourse.bacc as bacc
    import concourse.tile as tile
    from concourse import mybir

    bf16 = mybir.dt.bfloat16
    f32 = mybir.dt.float32
    AF = mybir.ActivationFunctionType

    nc = bacc.Bacc("TRN2", target_bir_lowering=False, debug=False)

    pairs = n_loc * C
    xt2_d = nc.dram_tensor("xt2", [pairs, W, H], bf16, kind="ExternalInput")
    cc_d = nc.dram_tensor("cc", [pairs, W, H], bf16, kind="ExternalInput")
    ee_d = nc.dram_tensor("ee", [pairs, H, W], bf16, kind="ExternalInput")
    lab_d = nc.dram_tensor("lab", [n_loc, H, W], bf16, kind="ExternalInput")
    id_d = nc.dram_tensor("ident", [128, 128], bf16, kind="ExternalInput")
    out_d = nc.dram_tensor("out", [128, 1], f32, kind="ExternalOutput")

    with ExitStack() as ctx:
        tc = ctx.enter_context(tile.TileContext(nc))
        loads = ctx.enter_context(tc.tile_pool(name="loads", bufs=13))
        spool = ctx.enter_context(tc.tile_pool(name="spool", bufs=4))
        tree = ctx.enter_context(tc.tile_pool(name="tree", bufs=4))
        small = ctx.enter_context(tc.tile_pool(name="small", bufs=4))
        singles = ctx.enter_context(tc.tile_pool(name="singles", bufs=1))
        psum = ctx.enter_context(tc.tile_pool(name="psum", bufs=2, space="PSUM"))

        ident_t = singles.tile([128, 128], bf16)
        nc.sync.dma_start(ident_t[:], id_d[:, :])
        partial = singles.tile([128, n_loc * MC], f32)

        for n in range(n_loc):
            xt2_ts, cc_ts, ee_ts = [], [], []
            for c in range(C):
                i = n * C + c
                xt2_t = loads.tile([128, KC, H], bf16, tag="xt2",
                                   name=f"xt2_{n}_{c}")
                nc.sync.dma_start(
                    xt2_t[:], xt2_d[i].rearrange("(kc p) h -> p kc h", p=128)
                )
                cc_t = loads.tile([128, KC, W], bf16, tag="cc",
                                  name=f"cc_{n}_{c}")
                nc.sync.dma_start(
                    cc_t[:], cc_d[i].rearrange("(kc p) w -> p kc w", p=128)
                )
                ee_t = loads.tile([128, MC, W], bf16, tag="ee",
                                  name=f"ee_{n}_{c}")
                nc.sync.dma_start(
                    ee_t[:], ee_d[i].rearrange("(mc p) w -> p mc w", p=128)
                )
                xt2_ts.append(xt2_t); cc_ts.append(cc_t); ee_ts.append(ee_t)

            for mc in range(MC):
                S = spool.tile([128, C, W], bf16, tag="S", name=f"S_{n}_{mc}")
                for g, (c0, gsz) in enumerate(GROUPS):
                    ps = psum.tile([128, gsz, W], f32,
                                   padded_shape=[128, 4, 512], tag="psg",
                                   name=f"ps_{n}_{mc}_{g}")
                    for j in range(gsz):
                        c = c0 + j
                        for kc in range(KC):
                            nc.tensor.matmul(
                                ps[:, j, :],
                                xt2_ts[c][:, kc, mc * 128 : (mc + 1) * 128],
                                cc_ts[c][:, kc, :],
                                start=(kc == 0),
                                stop=False,
                            )
                        nc.tensor.matmul(
                            ps[:, j, :], ident_t[:], ee_ts[c][:, mc, :],
                            start=False, stop=True,
                        )
                    # batched PSUM -> SBUF bf16 drain on the scalar engine
                    nc.scalar.copy(S[:, c0 : c0 + gsz, :], ps[:])

                # running max over classes: 5-way tree
                m5 = tree.tile([128, 5, W], bf16, tag="m5", name=f"m5_{n}_{mc}")
                nc.vector.tensor_max(m5[:], S[:, 0:5, :], S[:, 5:10, :])
                m2 = tree.tile([128, 2, W], bf16, tag="m2", name=f"m2_{n}_{mc}")
                nc.vector.tensor_max(m2[:], m5[:, 0:2, :], m5[:, 2:4, :])
                m = small.tile([128, W], bf16, tag="m", name=f"m_{n}_{mc}")
                nc.vector.tensor_max(m[:], m2[:, 0, :], m2[:, 1, :])
                nc.vector.tensor_max(m[:], m[:], m5[:, 4, :])
                nc.vector.tensor_max(m[:], m[:], S[:, 10, :])

                # d = s - m  (single broadcast subtract, in place over S)
                m_ap = m[:]
                m_b = bass.AP(
                    tensor=m_ap.tensor, offset=m_ap.offset,
                    ap=[list(m_ap.ap[0]), [0, C], list(m_ap.ap[1])],
                )
                nc.vector.tensor_sub(S[:], S[:], m_b)
                # e = exp(d), in place over S, one batched op
                nc.scalar.activation(S[:], S[:], AF.Exp)

                # acc = sum_c e  : 5-way tree
                a5 = tree.tile([128, 5, W], bf16, tag="a5", name=f"a5_{n}_{mc}")
                nc.vector.tensor_add(a5[:], S[:, 0:5, :], S[:, 5:10, :])
                a2 = tree.tile([128, 2, W], bf16, tag="a2", name=f"a2_{n}_{mc}")
                nc.vector.tensor_add(a2[:], a5[:, 0:2, :], a5[:, 2:4, :])
                acc = small.tile([128, W], bf16, tag="acc", name=f"acc_{n}_{mc}")
                nc.vector.tensor_add(acc[:], a2[:, 0, :], a2[:, 1, :])
                nc.vector.tensor_add(acc[:], acc[:], a5[:, 4, :])
                nc.vector.tensor_add(acc[:], acc[:], S[:, 10, :])

                labt = loads.tile([128, W], bf16, tag="lab", name=f"lab_{n}_{mc}")
                nc.sync.dma_start(labt[:], lab_d[n, mc * 128 : (mc + 1) * 128, :])
                w_t = small.tile([128, W], f32, tag="w", name=f"w_{n}_{mc}")
                if TAIL_MODE == "divide":
                    nc.vector.tensor_tensor(
                        out=w_t[:], in0=labt[:], in1=acc[:],
                        op=mybir.AluOpType.divide,
                    )
                elif TAIL_MODE == "gpsimd_divide":
                    nc.gpsimd.tensor_tensor(
                        out=w_t[:], in0=labt[:], in1=acc[:],
                        op=mybir.AluOpType.divide,
                    )
                else:
                    t = small.tile([128, W], f32, tag="t", name=f"t_{n}_{mc}")
                    nc.vector.reciprocal(t[:], acc[:])
                    nc.vector.tensor_mul(w_t[:], t[:], labt[:])
                slot = n * MC + mc
                nc.vector.tensor_reduce(
                    partial[:, slot : slot + 1],
                    w_t[:],
                    axis=mybir.AxisListType.X,
                    op=mybir.AluOpType.add,
                )

        pf = singles.tile([128, 1], f32)
        nc.vector.tensor_reduce(
            pf[:], partial[:], axis=mybir.AxisListType.X, op=mybir.AluOpType.add
        )
        nc.sync.dma_start(out_d[:, :], pf[:])

    nc.compile()
    return nc


def _get_compiled():
    global _COMPILED
    if _COMPILED is None:
        _COMPILED = _build()
    return _COMPILED


def _host_prep(x, centers, labels):
    x = np.asarray(x, dtype=np.float32)
    centers = np.asarray(centers, dtype=np.float32)
    labels_np = np.asarray(labels)

    n_zero = int((labels_np == 0).sum())

    xt2 = np.ascontiguousarray(
        np.transpose(-2.0 * x, (0, 1, 3, 2))
    ).astype(_BF16)                       # (N, C, W, H)
    cc = centers.astype(_BF16)            # (N, C, H, W)
    ee = (x * x + centers * centers).astype(_BF16)
    lab = labels_np.astype(np.float32).astype(_BF16)  # (N, H, W), values 0..10 exact
    ident = np.eye(128, dtype=_BF16)

    in_maps = []
    for core in range(N_CORES):
        sl = slice(core * N_LOC, (core + 1) * N_LOC)
        in_maps.append(
            {
                "xt2": np.ascontiguousarray(xt2[sl]).reshape(PAIRS, W, H),
                "cc": np.ascontiguousarray(cc[sl]).reshape(PAIRS, H, W),
                "ee": np.ascontiguousarray(ee[sl]).reshape(PAIRS, H, W),
                "lab": np.ascontiguousarray(lab[sl]),
                "ident": ident,
            }
        )
    return in_maps, n_zero


def kernel(x, centers, labels, _trace=False, _trace_kwargs=None):
    from concourse import bass_utils

    nc = _get_compiled()
    in_maps, n_zero = _host_prep(x, centers, labels)

    kwargs = {}
    if _trace:
        kwargs = dict(trace=True, **(_trace_kwargs or {}))
    res = bass_utils.run_bass_kernel_spmd(
        nc, in_maps, core_ids=list(range(N_CORES)), **kwargs
    )

    total = 0.0
    for core in range(N_CORES):
        total += float(res.results[core]["out"].astype(np.float64).sum())
    loss = (total + 1e-12 * n_zero) / float(N * H * W)
    out = np.float32(loss)
    if _trace:
        return out, res
    return out
